# revision 13
# baseline (speedup 1.0000x reference)
"""AdditiveAttention pooling kernel for 8 trn2 NeuronCores.

out = softmax(v @ tanh(h @ W.T + b).T) @ h   for h [131072, 768].

Strategy: shard rows of h across 8 cores. Each core makes a single pass over
its shard (shipped host-side as bf16, pre-transposed), computing unnormalized
weights a_i = exp(s_i - 10) and per-block partial weighted sums. The softmax
normalization happens on the host: out = sum P / sum a  (exact — scores are
bounded, so no running max is needed).

Per 512-row block, software-pipelined one block deep:
  PE : z^T = W^T.T @ h^T (36 bf16 matmuls); s = v.T @ tanh(z^T)
       (6 matmuls, emitted one block late so they never stall on ACT)
  ACT: tanh(z + b) with per-partition bias; exp(s - 10) with accum_out
       producing the per-block softmax denominator for free
  DMA: alpha row [1,512] bounced through DRAM and read back with a
       stride-0 broadcast AP -> ab [128,512] (replaces a PE broadcast
       matmul + DVE cast)
  DVE: fused weighted sum via scalar_tensor_tensor:
       P[m, blk] = sum_i ab[m, i] * hT[m, i]  in one instruction per chunk
Outputs are tiny: part [128, 6] f32 (block partials folded on-device) and
den [1, 32] f32 (per-block exp sums).
"""

import numpy as np
import ml_dtypes

import concourse.bass as bass
import concourse.tile as tile
from concourse import mybir
from concourse.bass_utils import run_bass_kernel_spmd
from concourse.vector_clock import ScopedClock, VectorClock

N = 131072
H = 768
NCORES = 8
SHARD = N // NCORES          # 16384 rows per core
R = 512                      # rows per block
NB = SHARD // R              # 32 blocks
MC = H // 128                # 6 chunks of the feature dim
EXP_SHIFT = -10.0            # exp(s - 10): keeps a_i comfortably in bf16

BF16 = mybir.dt.bfloat16
F16 = mybir.dt.float16
F32 = mybir.dt.float32
NPBF16 = ml_dtypes.bfloat16

_ENGINE_CACHE = {}


def _patch_tail_drain():
    """This walrus build rejects instructions carrying >1 semaphore wait.

    1) Tile's end-of-context drain aggregates one wait per live processor
       onto a single SP Drain — split into one SP nop per pending processor
       tick, then a wait-free drain (same engine + program order: equivalent).
    2) Tile's wait assignment can attach 2-3 waits to body instructions.
       Before lowering, hoist all but one wait of each instruction onto
       same-engine NOPs inserted right before it (engine queues are FIFO)."""
    if getattr(tile.TileContext, "_ant_tail_patch", False):
        return

    def _drain_and_barrier(self, tick_clock, wait_clock):
        gvc = tick_clock.global_clock
        n = len(gvc)
        for p in range(n):
            t = gvc[p]
            if t > 0:
                req = [0] * n
                req[p] = t
                nop = self.nc.sync.nop()
                wait_clock.add_sem_waits(
                    nop.ins, ScopedClock({None: VectorClock(req)})
                )
        self.nc.sync.drain()
        self.nc.all_engine_barrier()
        popped = self.nc._tile_sem_poison_stack.pop()
        assert popped is self._sem_poison
        self.nc.clear_and_free_semaphores(list(self.sems.allocated().values()))
        self.nc.all_engine_barrier()

    tile.TileContext._drain_and_barrier = _drain_and_barrier

    orig_lower = tile.TileContext._lower_ordered_insts

    def _lower_with_wait_split(self, ordered):
        for insts in ordered.values():
            out = []
            for inst in insts:
                si = getattr(inst, "sync_info", None)
                if si is not None and len(si.on_wait) > 1:
                    waits = list(si.on_wait)
                    eng = inst.engine
                    for w in waits[:-1]:
                        nop = mybir.InstNoOp(
                            name=self.nc.get_next_instruction_name(),
                            ins=[],
                            outs=[],
                            engine=eng,
                        )
                        nop.sync_info = mybir.SyncInfo(on_wait=[w], on_update=[])
                        out.append(nop)
                    inst.sync_info = mybir.SyncInfo(
                        on_wait=[waits[-1]], on_update=list(si.on_update)
                    )
                out.append(inst)
            insts[:] = out
        return orig_lower(self, ordered)

    tile.TileContext._lower_ordered_insts = _lower_with_wait_split
    tile.TileContext._ant_tail_patch = True


def build_kernel():
    _patch_tail_drain()
    nc = bass.Bass("TRN2", debug=False)

    hT = nc.dram_tensor("hT16", [H, SHARD], BF16, kind="ExternalInput").ap()
    wT = nc.dram_tensor("WT16", [H, H], BF16, kind="ExternalInput").ap()
    bT = nc.dram_tensor("bT", [128, MC], F32, kind="ExternalInput").ap()
    vT = nc.dram_tensor("vT", [128, MC], BF16, kind="ExternalInput").ap()
    ident = nc.dram_tensor("ident", [128, 128], F16, kind="ExternalInput").ap()
    bounce = nc.dram_tensor("bounce", [NB, R], BF16, kind="Internal").ap()
    part_out = nc.dram_tensor("part", [128, MC], F32, kind="ExternalOutput").ap()
    den_out = nc.dram_tensor("den", [4, NB], F32, kind="ExternalOutput").ap()

    with tile.TileContext(nc) as tc:
        with tc.tile_pool(name="consts", bufs=1) as consts, \
             tc.tile_pool(name="hTp", bufs=5) as hT_pool, \
             tc.tile_pool(name="uTp", bufs=8) as uT_pool, \
             tc.tile_pool(name="arow", bufs=2) as arow_pool, \
             tc.tile_pool(name="scp", bufs=2) as sc_pool, \
             tc.tile_pool(name="abp", bufs=3) as ab_pool, \
             tc.tile_pool(name="ttout", bufs=2) as tt_pool, \
             tc.tile_pool(name="zps", bufs=3, space="PSUM") as z_pool, \
             tc.tile_pool(name="sps", bufs=2, space="PSUM") as s_pool, \
             tc.tile_pool(name="a4ps", bufs=2, space="PSUM") as a4_pool, \
             tc.tile_pool(name="wps", bufs=1, space="PSUM") as warm_pool, \
             tc.tile_pool(name="outp", bufs=1) as out_pool:

            # --- constants loaded once (scalar-engine trigger: its own
            # DMA queue, so the W load doesn't queue behind block 0's hT).
            # One tile per jc column so block 0's first z-group only waits
            # for its own sixth of W. ---
            wT_sbs = []
            for jc in range(MC):
                w_t = consts.tile([128, MC * 128], BF16, tag=f"w{jc}")
                nc.scalar.dma_start(
                    w_t[:].rearrange("p (c j) -> p c j", c=MC),
                    wT[:, jc * 128 : (jc + 1) * 128].rearrange(
                        "(c p) j -> p c j", p=128
                    ),
                )
                wT_sbs.append(w_t)
            bT_sb = consts.tile([128, MC], F32)
            nc.scalar.dma_start(bT_sb[:], bT)
            vT_sb = consts.tile([128, MC], BF16)
            nc.scalar.dma_start(vT_sb[:], vT)
            id_sb = consts.tile([128, 128], F16)
            nc.scalar.dma_start(id_sb[:], ident)
            shift_sb = consts.tile([4, 1], F32)
            nc.vector.memset(shift_sb[:], EXP_SHIFT)

            den_sb = out_pool.tile([4, NB], F32)
            pcol_sb = out_pool.tile([128, MC * NB], F32)  # [p, mc*NB + blk]
            part_sb = out_pool.tile([128, MC], F32)

            # Warm-up while the first DMAs are in flight: ~3.5us of dummy
            # matmuls flips the PE HAM clock gate to 2.4 GHz before real work
            # arrives, and a tiny tanh+exp pulls the ACT table loads off the
            # critical path. All operate on a zeroed scratch tile.
            warm_sb = consts.tile([128, 512], BF16)
            nc.vector.memset(warm_sb[:], 0.0)
            warm_ps = warm_pool.tile([128, R], F32, tag="warm")
            for wi in range(8):
                nc.tensor.matmul(
                    warm_ps[:],
                    lhsT=warm_sb[:, 0:128],
                    rhs=warm_sb[:],
                    start=(wi == 0),
                    stop=(wi == 7),
                    skip_group_check=True,
                )
            warm_u = uT_pool.tile([128, 32], BF16, tag="warm_u")
            nc.scalar.activation(
                warm_u[:], warm_ps[:, 0:32],
                mybir.ActivationFunctionType.Tanh, bias=0.0, scale=1.0,
            )
            nc.scalar.activation(
                warm_u[:], warm_ps[:, 0:32],
                mybir.ActivationFunctionType.Exp, bias=0.0, scale=1.0,
            )

            # per-block live state for the software pipeline
            state = {}

            def emit_s_batch(blk):
                """Scores for block blk via u-stationary matmuls: 24 1-col
                matmuls (28ns each) replace 6 512-wide streams, then one PE
                transpose turns the [128,4] score columns into a [4,128]
                row-major tile."""
                st = state[blk]
                s_col = s_pool.tile([128, 4], F32)
                # ic outer: accumulation groups must be sequential — this
                # hardware corrupts interleaved groups within one PSUM bank
                for ic in range(4):
                    for jc in range(MC):
                        nc.tensor.matmul(
                            s_col[:, ic : ic + 1],
                            lhsT=st["u"][jc][:, ic * 128 : (ic + 1) * 128],
                            rhs=vT_sb[:, jc : jc + 1],
                            start=(jc == 0),
                            stop=(jc == MC - 1),
                            skip_group_check=True,
                        )
                sc16 = sc_pool.tile([128, 4], F16)
                nc.vector.tensor_copy(sc16[:], s_col[:])
                a4_ps = a4_pool.tile([4, 128], F16)
                nc.tensor.transpose(a4_ps[:], sc16[:], id_sb[:])
                st["a4_ps"] = a4_ps

            def emit_exp_bounce(blk):
                """exp(s-10) + accum denominators, then DRAM-bounce bcast."""
                st = state[blk]
                a4row = arow_pool.tile([4, 128], BF16)
                nc.scalar.activation(
                    a4row[:], st["a4_ps"][:], mybir.ActivationFunctionType.Exp,
                    bias=shift_sb[:], scale=1.0,
                    accum_out=den_sb[:, blk : blk + 1],
                )
                nc.sync.dma_start(
                    bounce[blk : blk + 1, :].rearrange("b (c q) -> (b c) q", c=4),
                    a4row[:],
                )
                ab = ab_pool.tile([128, R], BF16)
                src = bass.AP(
                    tensor=bounce.tensor,
                    offset=bounce.offset + blk * R,
                    ap=[[0, 128], [128, 4], [1, 128]],
                )
                nc.sync.dma_start(
                    ab[:].rearrange("p (c q) -> p c q", c=4), src
                )
                st["ab"] = ab

            def emit_ws(blk):
                """fused weighted sum: pcol[:, mc*NB+blk] = sum_i ab*hT."""
                st = state[blk]
                for mc in range(MC):
                    tt = tt_pool.tile([128, R], BF16)
                    nc.vector.scalar_tensor_tensor(
                        out=tt[:],
                        in0=st["hT"][:, mc * R : (mc + 1) * R],
                        scalar=1.0,
                        in1=st["ab"][:],
                        op0=mybir.AluOpType.mult,
                        op1=mybir.AluOpType.mult,
                        accum_out=pcol_sb[:, mc * NB + blk : mc * NB + blk + 1],
                    )
                del state[blk]["hT"], state[blk]["ab"], state[blk]["u"]

            for blk in range(NB):
                hT_t = hT_pool.tile([128, MC * R], BF16)
                nc.gpsimd.dma_start(
                    hT_t[:].rearrange("p (c i) -> p c i", c=MC),
                    hT[:, blk * R : (blk + 1) * R].rearrange(
                        "(c p) i -> p c i", p=128
                    ),
                )
                state[blk] = {"hT": hT_t, "u": {}}

                for jc in range(MC):
                    z_ps = z_pool.tile([128, R], F32)
                    for mc in range(MC):
                        nc.tensor.matmul(
                            z_ps[:],
                            lhsT=wT_sbs[jc][:, mc * 128 : (mc + 1) * 128],
                            rhs=hT_t[:, mc * R : (mc + 1) * R],
                            start=(mc == 0),
                            stop=(mc == MC - 1),
                        )
                    uT_t = uT_pool.tile([128, R], BF16)
                    nc.scalar.activation(
                        uT_t[:], z_ps[:], mybir.ActivationFunctionType.Tanh,
                        bias=bT_sb[:, jc : jc + 1], scale=1.0,
                    )
                    state[blk]["u"][jc] = uT_t

                    # pipelined tail work for earlier blocks, slotted between
                    # z-groups so PE/ACT never stall on each other
                    if jc == 0 and blk >= 1:
                        emit_s_batch(blk - 1)
                    elif jc == 1 and blk >= 1:
                        emit_exp_bounce(blk - 1)
                    elif jc == 4 and blk >= 1:
                        emit_ws(blk - 1)

            # drain the pipeline
            emit_s_batch(NB - 1)
            emit_exp_bounce(NB - 1)
            emit_ws(NB - 1)

            # fold block partials: part[:, mc] = sum_blk pcol[:, mc*NB+blk]
            for mc in range(MC):
                nc.vector.tensor_reduce(
                    part_sb[:, mc : mc + 1],
                    pcol_sb[:, mc * NB : (mc + 1) * NB],
                    axis=mybir.AxisListType.X,
                    op=mybir.AluOpType.add,
                )

            nc.gpsimd.dma_start(part_out, part_sb[:])
            nc.gpsimd.dma_start(den_out, den_sb[:])

    return nc


def _get_engine():
    if "nc" not in _ENGINE_CACHE:
        _ENGINE_CACHE["nc"] = build_kernel()
    return _ENGINE_CACHE["nc"]


def make_in_maps(inputs):
    h_i = np.asarray(inputs["h_i"])
    W_weight = np.asarray(inputs["W_weight"])
    W_bias = np.asarray(inputs["W_bias"])
    v = np.asarray(inputs["v"])

    hT16 = np.ascontiguousarray(h_i.astype(NPBF16).T)
    wT16 = np.ascontiguousarray(W_weight.T.astype(NPBF16))
    bT = np.ascontiguousarray(W_bias.astype(np.float32).reshape(MC, 128).T)
    vT = np.ascontiguousarray(v.reshape(MC, 128).T.astype(NPBF16))
    ident = np.eye(128, dtype=np.float16)

    in_maps = []
    for c in range(NCORES):
        r0, r1 = c * SHARD, (c + 1) * SHARD
        in_maps.append({
            "hT16": np.ascontiguousarray(hT16[:, r0:r1]),
            "WT16": wT16,
            "bT": bT,
            "vT": vT,
            "ident": ident,
        })
    return in_maps


def kernel(h_i, W_weight, W_bias, v, trace=False):
    in_maps = make_in_maps(
        {"h_i": h_i, "W_weight": W_weight, "W_bias": W_bias, "v": v}
    )
    nc = _get_engine()
    res = run_bass_kernel_spmd(
        nc, in_maps, core_ids=list(range(NCORES)), trace=trace
    )
    _ENGINE_CACHE["last_results"] = res

    num = np.zeros(H, dtype=np.float64)
    den = 0.0
    for c in range(NCORES):
        # part [128, MC]: element [p, mc] is the shard partial for
        # feature m = mc*128 + p
        part = res.results[c]["part"].astype(np.float64)
        num += part.T.reshape(H)
        den += res.results[c]["den"].astype(np.float64).sum()
    out = (num / den).astype(np.float32).reshape(1, H)
    return out


if __name__ == "__main__":
    rng = np.random.default_rng(0)
    h = rng.standard_normal((N, H), dtype=np.float32)
    W = (rng.standard_normal((H, H)) * 0.02).astype(np.float32)
    b = (rng.standard_normal(H) * 0.02).astype(np.float32)
    vv = (rng.standard_normal((1, H)) * 0.1).astype(np.float32)
    out = kernel(h, W, b, vv)
    u = np.tanh(h.astype(np.float64) @ W.astype(np.float64).T + b)
    s = (vv.astype(np.float64) @ u.T).ravel()
    a = np.exp(s - s.max())
    ref = (a @ h.astype(np.float64)) / a.sum()
    rel = np.linalg.norm(out.ravel() - ref) / np.linalg.norm(ref)
    print("rel err vs fp64 numpy ref:", rel)


# revision 20
# speedup vs baseline: 1.0725x; 1.0725x over previous
"""AdditiveAttention pooling kernel for 8 trn2 NeuronCores.

out = softmax(v @ tanh(h @ W.T + b).T) @ h   for h [131072, 768].

Strategy: shard rows of h across 8 cores. Each core makes a single pass over
its shard (shipped host-side as bf16, pre-transposed), computing unnormalized
weights a_i = exp(s_i - 10) and per-block partial weighted sums. The softmax
normalization happens on the host: out = sum P / sum a  (exact — scores are
bounded, so no running max is needed).

Per 512-row block, software-pipelined one block deep:
  PE : z^T = W^T.T @ h^T (36 bf16 matmuls); s = v.T @ tanh(z^T)
       (6 matmuls, emitted one block late so they never stall on ACT)
  ACT: tanh(z + b) with per-partition bias; exp(s - 10) with accum_out
       producing the per-block softmax denominator for free
  DMA: alpha row [1,512] bounced through DRAM and read back with a
       stride-0 broadcast AP -> ab [128,512] (replaces a PE broadcast
       matmul + DVE cast)
  DVE: fused weighted sum via scalar_tensor_tensor:
       P[m, blk] = sum_i ab[m, i] * hT[m, i]  in one instruction per chunk
Outputs are tiny: part [128, 6] f32 (block partials folded on-device) and
den [1, 32] f32 (per-block exp sums).
"""

import numpy as np
import ml_dtypes

import concourse.bass as bass
import concourse.tile as tile
from concourse import mybir
from concourse.bass_utils import run_bass_kernel_spmd
from concourse.vector_clock import ScopedClock, VectorClock

N = 131072
H = 768
NCORES = 8
SHARD = N // NCORES          # 16384 rows per core
R = 512                      # rows per block
NB = SHARD // R              # 32 blocks
MC = H // 128                # 6 chunks of the feature dim
EXP_SHIFT = -10.0            # exp(s - 10): keeps a_i comfortably in bf16

BF16 = mybir.dt.bfloat16
F16 = mybir.dt.float16
F32 = mybir.dt.float32
NPBF16 = ml_dtypes.bfloat16

_ENGINE_CACHE = {}


def _patch_tail_drain():
    """This walrus build rejects instructions carrying >1 semaphore wait.

    1) Tile's end-of-context drain aggregates one wait per live processor
       onto a single SP Drain — split into one SP nop per pending processor
       tick, then a wait-free drain (same engine + program order: equivalent).
    2) Tile's wait assignment can attach 2-3 waits to body instructions.
       Before lowering, hoist all but one wait of each instruction onto
       same-engine NOPs inserted right before it (engine queues are FIFO)."""
    if getattr(tile.TileContext, "_ant_tail_patch", False):
        return

    def _drain_and_barrier(self, tick_clock, wait_clock):
        gvc = tick_clock.global_clock
        n = len(gvc)
        for p in range(n):
            t = gvc[p]
            if t > 0:
                req = [0] * n
                req[p] = t
                nop = self.nc.sync.nop()
                wait_clock.add_sem_waits(
                    nop.ins, ScopedClock({None: VectorClock(req)})
                )
        self.nc.sync.drain()
        self.nc.all_engine_barrier()
        popped = self.nc._tile_sem_poison_stack.pop()
        assert popped is self._sem_poison
        self.nc.clear_and_free_semaphores(list(self.sems.allocated().values()))
        self.nc.all_engine_barrier()

    tile.TileContext._drain_and_barrier = _drain_and_barrier

    orig_lower = tile.TileContext._lower_ordered_insts

    def _lower_with_wait_split(self, ordered):
        for insts in ordered.values():
            out = []
            for inst in insts:
                si = getattr(inst, "sync_info", None)
                if si is not None and len(si.on_wait) > 1:
                    waits = list(si.on_wait)
                    eng = inst.engine
                    for w in waits[:-1]:
                        nop = mybir.InstNoOp(
                            name=self.nc.get_next_instruction_name(),
                            ins=[],
                            outs=[],
                            engine=eng,
                        )
                        nop.sync_info = mybir.SyncInfo(on_wait=[w], on_update=[])
                        out.append(nop)
                    inst.sync_info = mybir.SyncInfo(
                        on_wait=[waits[-1]], on_update=list(si.on_update)
                    )
                out.append(inst)
            insts[:] = out
        return orig_lower(self, ordered)

    tile.TileContext._lower_ordered_insts = _lower_with_wait_split
    tile.TileContext._ant_tail_patch = True


def build_kernel():
    _patch_tail_drain()
    nc = bass.Bass("TRN2", debug=False)

    hT = nc.dram_tensor("hT16", [H, SHARD], BF16, kind="ExternalInput").ap()
    wT = nc.dram_tensor("WT16", [H, H], BF16, kind="ExternalInput").ap()
    bT = nc.dram_tensor("bT", [128, MC], F32, kind="ExternalInput").ap()
    vT = nc.dram_tensor("vT", [128, MC], BF16, kind="ExternalInput").ap()
    ident = nc.dram_tensor("ident", [128, 128], F16, kind="ExternalInput").ap()
    bounce = nc.dram_tensor("bounce", [NB, R], BF16, kind="Internal").ap()
    part_out = nc.dram_tensor("part", [128, MC], F32, kind="ExternalOutput").ap()
    den_out = nc.dram_tensor("den", [4, NB], F32, kind="ExternalOutput").ap()

    with tile.TileContext(nc) as tc:
        with tc.tile_pool(name="consts", bufs=1) as consts, \
             tc.tile_pool(name="hTp", bufs=5) as hT_pool, \
             tc.tile_pool(name="h0p", bufs=1) as h0_pool, \
             tc.tile_pool(name="uTp", bufs=8) as uT_pool, \
             tc.tile_pool(name="arow", bufs=2) as arow_pool, \
             tc.tile_pool(name="scp", bufs=2) as sc_pool, \
             tc.tile_pool(name="abp", bufs=3) as ab_pool, \
             tc.tile_pool(name="ttout", bufs=2) as tt_pool, \
             tc.tile_pool(name="zps", bufs=3, space="PSUM") as z_pool, \
             tc.tile_pool(name="sps", bufs=2, space="PSUM") as s_pool, \
             tc.tile_pool(name="a4ps", bufs=2, space="PSUM") as a4_pool, \
             tc.tile_pool(name="wps", bufs=1, space="PSUM") as warm_pool, \
             tc.tile_pool(name="outp", bufs=1) as out_pool:

            # --- constants loaded once (scalar-engine trigger: its own
            # DMA queue, so the W load doesn't queue behind block 0's hT).
            # One tile per jc column so block 0's first z-group only waits
            # for its own sixth of W. ---
            wT_sbs = []
            for jc in range(MC):
                w_t = consts.tile([128, MC * 128], BF16, tag=f"w{jc}")
                nc.scalar.dma_start(
                    w_t[:].rearrange("p (c j) -> p c j", c=MC),
                    wT[:, jc * 128 : (jc + 1) * 128].rearrange(
                        "(c p) j -> p c j", p=128
                    ),
                )
                wT_sbs.append(w_t)
            bT_sb = consts.tile([128, MC], F32)
            nc.scalar.dma_start(bT_sb[:], bT)
            vT_sb = consts.tile([128, MC], BF16)
            nc.scalar.dma_start(vT_sb[:], vT)
            id_sb = consts.tile([128, 128], F16)
            nc.scalar.dma_start(id_sb[:], ident)
            shift_sb = consts.tile([4, 1], F32)
            nc.vector.memset(shift_sb[:], EXP_SHIFT)

            den_sb = out_pool.tile([4, NB], F32)
            pcol_sb = out_pool.tile([128, MC * NB], F32)  # [p, mc*NB + blk]
            part_sb = out_pool.tile([128, MC], F32)

            # Warm-up while the first DMAs are in flight: ~3.5us of dummy
            # matmuls flips the PE HAM clock gate to 2.4 GHz before real work
            # arrives, and a tiny tanh+exp pulls the ACT table loads off the
            # critical path. All operate on a zeroed scratch tile.
            warm_sb = consts.tile([128, 512], BF16)
            nc.vector.memset(warm_sb[:], 0.0)
            warm_ps = warm_pool.tile([128, R], F32, tag="warm")
            NWARM = 20  # ~8.5us: bridges the DMA wait for block 0 so the
            #             PE p-state ramp finishes before real work arrives
            for wi in range(NWARM):
                nc.tensor.matmul(
                    warm_ps[:],
                    lhsT=warm_sb[:, 0:128],
                    rhs=warm_sb[:],
                    start=(wi == 0),
                    stop=(wi == NWARM - 1),
                    skip_group_check=True,
                )
            warm_u = uT_pool.tile([128, 32], BF16, tag="warm_u")
            nc.scalar.activation(
                warm_u[:], warm_ps[:, 0:32],
                mybir.ActivationFunctionType.Tanh, bias=0.0, scale=1.0,
            )
            nc.scalar.activation(
                warm_u[:], warm_ps[:, 0:32],
                mybir.ActivationFunctionType.Exp, bias=0.0, scale=1.0,
            )

            # per-block live state for the software pipeline
            state = {}

            def emit_s_batch(blk):
                """Scores for block blk via u-stationary matmuls: 24 1-col
                matmuls (28ns each) replace 6 512-wide streams, then one PE
                transpose turns the [128,4] score columns into a [4,128]
                row-major tile."""
                st = state[blk]
                s_col = s_pool.tile([128, 4], F32)
                # ic outer: accumulation groups must be sequential — this
                # hardware corrupts interleaved groups within one PSUM bank
                for ic in range(4):
                    for jc in range(MC):
                        nc.tensor.matmul(
                            s_col[:, ic : ic + 1],
                            lhsT=st["u"][jc][:, ic * 128 : (ic + 1) * 128],
                            rhs=vT_sb[:, jc : jc + 1],
                            start=(jc == 0),
                            stop=(jc == MC - 1),
                            skip_group_check=True,
                        )
                sc16 = sc_pool.tile([128, 4], F16)
                nc.vector.tensor_copy(sc16[:], s_col[:])
                st["sc16"] = sc16

            def emit_transpose(blk):
                st = state[blk]
                a4_ps = a4_pool.tile([4, 128], F16)
                nc.tensor.transpose(a4_ps[:], st["sc16"][:], id_sb[:])
                st["a4_ps"] = a4_ps

            def emit_exp_bounce(blk):
                """exp(s-10) + accum denominators, then DRAM-bounce bcast."""
                st = state[blk]
                a4row = arow_pool.tile([4, 128], BF16)
                nc.scalar.activation(
                    a4row[:], st["a4_ps"][:], mybir.ActivationFunctionType.Exp,
                    bias=shift_sb[:], scale=1.0,
                    accum_out=den_sb[:, blk : blk + 1],
                )
                nc.sync.dma_start(
                    bounce[blk : blk + 1, :].rearrange("b (c q) -> (b c) q", c=4),
                    a4row[:],
                )
                ab = ab_pool.tile([128, R], BF16)
                src = bass.AP(
                    tensor=bounce.tensor,
                    offset=bounce.offset + blk * R,
                    ap=[[0, 128], [128, 4], [1, 128]],
                )
                nc.sync.dma_start(
                    ab[:].rearrange("p (c q) -> p c q", c=4), src
                )
                st["ab"] = ab

            def emit_ws(blk):
                """fused weighted sum: pcol[:, mc*NB+blk] = sum_i ab*hT."""
                st = state[blk]
                for mc in range(MC):
                    tt = tt_pool.tile([128, R], BF16)
                    nc.vector.scalar_tensor_tensor(
                        out=tt[:],
                        in0=hchunk(st, mc),
                        scalar=1.0,
                        in1=st["ab"][:],
                        op0=mybir.AluOpType.mult,
                        op1=mybir.AluOpType.mult,
                        accum_out=pcol_sb[:, mc * NB + blk : mc * NB + blk + 1],
                    )
                state[blk] = {}

            def hchunk(st, mc):
                if "hT_chunks" in st:
                    return st["hT_chunks"][mc][:]
                return st["hT"][:, mc * R : (mc + 1) * R]

            for blk in range(NB):
                if blk == 0:
                    # block 0: one DMA per mc chunk, alternating trigger
                    # engines, so the first z-group starts ~5us earlier
                    chunks = []
                    for mc in range(MC):
                        t = h0_pool.tile([128, R], BF16, tag=f"h0{mc}")
                        eng = nc.gpsimd if mc % 2 == 0 else nc.sync
                        eng.dma_start(
                            t[:], hT[mc * 128 : (mc + 1) * 128, 0:R]
                        )
                        chunks.append(t)
                    state[blk] = {"hT_chunks": chunks, "u": {}}
                else:
                    hT_t = hT_pool.tile([128, MC * R], BF16)
                    nc.gpsimd.dma_start(
                        hT_t[:].rearrange("p (c i) -> p c i", c=MC),
                        hT[:, blk * R : (blk + 1) * R].rearrange(
                            "(c p) i -> p c i", p=128
                        ),
                    )
                    state[blk] = {"hT": hT_t, "u": {}}

                for jc in range(MC):
                    z_ps = z_pool.tile([128, R], F32)
                    for mc in range(MC):
                        nc.tensor.matmul(
                            z_ps[:],
                            lhsT=wT_sbs[jc][:, mc * 128 : (mc + 1) * 128],
                            rhs=hchunk(state[blk], mc),
                            start=(mc == 0),
                            stop=(mc == MC - 1),
                        )
                    uT_t = uT_pool.tile([128, R], BF16)
                    nc.scalar.activation(
                        uT_t[:], z_ps[:], mybir.ActivationFunctionType.Tanh,
                        bias=bT_sb[:, jc : jc + 1], scale=1.0,
                    )
                    state[blk]["u"][jc] = uT_t

                    # pipelined tail work for earlier blocks, slotted between
                    # z-groups so no engine stalls on another
                    if jc == 0 and blk >= 1:
                        emit_s_batch(blk - 1)
                    elif jc == 1 and blk >= 1:
                        emit_transpose(blk - 1)
                    elif jc == 2 and blk >= 1:
                        emit_exp_bounce(blk - 1)
                    elif jc == 4 and blk >= 1:
                        emit_ws(blk - 1)

            # drain the pipeline
            emit_s_batch(NB - 1)
            emit_transpose(NB - 1)
            emit_exp_bounce(NB - 1)
            emit_ws(NB - 1)

            # fold block partials: part[:, mc] = sum_blk pcol[:, mc*NB+blk]
            for mc in range(MC):
                nc.vector.tensor_reduce(
                    part_sb[:, mc : mc + 1],
                    pcol_sb[:, mc * NB : (mc + 1) * NB],
                    axis=mybir.AxisListType.X,
                    op=mybir.AluOpType.add,
                )

            nc.gpsimd.dma_start(part_out, part_sb[:])
            nc.gpsimd.dma_start(den_out, den_sb[:])

    return nc


def _get_engine():
    if "nc" not in _ENGINE_CACHE:
        _ENGINE_CACHE["nc"] = build_kernel()
    return _ENGINE_CACHE["nc"]


def make_in_maps(inputs):
    h_i = np.asarray(inputs["h_i"])
    W_weight = np.asarray(inputs["W_weight"])
    W_bias = np.asarray(inputs["W_bias"])
    v = np.asarray(inputs["v"])

    hT16 = np.ascontiguousarray(h_i.astype(NPBF16).T)
    wT16 = np.ascontiguousarray(W_weight.T.astype(NPBF16))
    bT = np.ascontiguousarray(W_bias.astype(np.float32).reshape(MC, 128).T)
    vT = np.ascontiguousarray(v.reshape(MC, 128).T.astype(NPBF16))
    ident = np.eye(128, dtype=np.float16)

    in_maps = []
    for c in range(NCORES):
        r0, r1 = c * SHARD, (c + 1) * SHARD
        in_maps.append({
            "hT16": np.ascontiguousarray(hT16[:, r0:r1]),
            "WT16": wT16,
            "bT": bT,
            "vT": vT,
            "ident": ident,
        })
    return in_maps


def kernel(h_i, W_weight, W_bias, v, trace=False):
    in_maps = make_in_maps(
        {"h_i": h_i, "W_weight": W_weight, "W_bias": W_bias, "v": v}
    )
    nc = _get_engine()
    res = run_bass_kernel_spmd(
        nc, in_maps, core_ids=list(range(NCORES)), trace=trace
    )
    _ENGINE_CACHE["last_results"] = res

    num = np.zeros(H, dtype=np.float64)
    den = 0.0
    for c in range(NCORES):
        # part [128, MC]: element [p, mc] is the shard partial for
        # feature m = mc*128 + p
        part = res.results[c]["part"].astype(np.float64)
        num += part.T.reshape(H)
        den += res.results[c]["den"].astype(np.float64).sum()
    out = (num / den).astype(np.float32).reshape(1, H)
    return out


if __name__ == "__main__":
    rng = np.random.default_rng(0)
    h = rng.standard_normal((N, H), dtype=np.float32)
    W = (rng.standard_normal((H, H)) * 0.02).astype(np.float32)
    b = (rng.standard_normal(H) * 0.02).astype(np.float32)
    vv = (rng.standard_normal((1, H)) * 0.1).astype(np.float32)
    out = kernel(h, W, b, vv)
    u = np.tanh(h.astype(np.float64) @ W.astype(np.float64).T + b)
    s = (vv.astype(np.float64) @ u.T).ravel()
    a = np.exp(s - s.max())
    ref = (a @ h.astype(np.float64)) / a.sum()
    rel = np.linalg.norm(out.ravel() - ref) / np.linalg.norm(ref)
    print("rel err vs fp64 numpy ref:", rel)
